# revision 1
# baseline (speedup 1.0000x reference)
"""Trainium2 Bass kernel for CvtLstm first-step (h=c=0) — full-IO contract.

Sharding: data-parallel, one batch sample per NeuronCore (N=8, 8 cores).

Per-core math (sample n), all matmuls in float32r (full-rate fp32 PE mode):
  q  = conv3x3_same (x, Wq_eff)   with Wq_eff = w_qx . w_in  (host-folded 1x1)
  k  = conv3x3_valid(x, Wk_eff)
  vT = conv3x3_valid(x, Wv_eff) produced directly transposed [D, A]
  sT[d,q] = k.q, two heads row-packed per PSUM unit (scoresT layout)
  expT = exp(sT)  (no max-subtraction: |s| bounded ~40, fp32 safe)
  AV with an appended ones-column in vT gives both a_raw and Z = sum_d exp
  a = a_raw / Z  (reciprocal + PE-replicated broadcast), gates = 1x1 convs,
  c = sig(gi)*tanh(gg), hn = sig(go)*tanh(c), out = 1x1 conv + b_out.

Structured to minimize synchronization edges (big PSUM accumulation groups,
merged ACT calls, batched partition-remap DMAs).
"""
import sys
import numpy as np

sys.path.insert(0, '/opt/trn_rl_repo')

import concourse.bass as bass  # noqa: E402
import concourse.tile as tile  # noqa: E402
from concourse import bacc, mybir  # noqa: E402
from concourse.bass_utils import run_bass_kernel_spmd  # noqa: E402

F32 = mybir.dt.float32
F32R = mybir.dt.float32r
AF = mybir.ActivationFunctionType

N, I, R, A, HEADS, O, H, W = 8, 128, 256, 256, 8, 256, 36, 36
HC = A // HEADS            # 32
Q = H * W                  # 1296
HK = H - 2                 # 34
D = HK * HK                # 1156
G = 3 * R                  # 768 gate channels kept (gi, gg, go)
HP = 38                    # padded width
XPADF = 39 * 38            # padded free size (extra row of slack)

Q_CHUNKS = [(0, 512), (512, 512), (1024, 272)]          # contiguous-free ops
QR_CHUNKS = [(0, 12), (12, 12), (24, 12)]               # q-conv row chunks
KR_CHUNKS = [(0, 12), (12, 12), (24, 10)]               # k-conv row chunks
DT = 10                                                 # 128-row d tiles
T2 = 12                                                 # 102-row vT conv tiles

_CACHE = {}


def _build(rep=1, phases=('conv', 'attn', 'norm')):
    nc = bacc.Bacc("TRN2", target_bir_lowering=False, debug=False)

    xc_d = nc.dram_tensor("xc", [128, Q], F32, kind="ExternalInput").ap()
    wq_d = nc.dram_tensor("wq", [128, 9, 256], F32, kind="ExternalInput").ap()
    wk_d = nc.dram_tensor("wk", [128, 9, 256], F32, kind="ExternalInput").ap()
    wv_d = nc.dram_tensor("wv", [128, 9, 256], F32, kind="ExternalInput").ap()
    wga_d = nc.dram_tensor("wga", [2, 128, G], F32, kind="ExternalInput").ap()
    wgx_d = nc.dram_tensor("wgx", [128, G], F32, kind="ExternalInput").ap()
    wout_d = nc.dram_tensor("wout", [2, 128, 256], F32,
                            kind="ExternalInput").ap()
    bg_d = nc.dram_tensor("bg", [128, 6], F32, kind="ExternalInput").ap()
    bo_d = nc.dram_tensor("bo", [128, 2], F32, kind="ExternalInput").ap()
    e_d = nc.dram_tensor("e", [8, 256], F32, kind="ExternalInput").ap()
    o_d = nc.dram_tensor("o", [256, Q], F32, kind="ExternalOutput").ap()

    with tile.TileContext(nc) as tc:
        with (
            tc.tile_pool(name="consts", bufs=1) as consts,
            tc.tile_pool(name="big", bufs=1) as big,
        ):
            wga_t = consts.tile([128, 2, G], F32R)
            nc.sync.dma_start(wga_t[:, 0, :], wga_d[0].bitcast(F32R))
            nc.sync.dma_start(wga_t[:, 1, :], wga_d[1].bitcast(F32R))
            wgx_t = consts.tile([128, G], F32R)
            nc.sync.dma_start(wgx_t[:], wgx_d.bitcast(F32R))
            wout_t = consts.tile([128, 2, 256], F32R)
            nc.sync.dma_start(wout_t[:, 0, :], wout_d[0].bitcast(F32R))
            nc.sync.dma_start(wout_t[:, 1, :], wout_d[1].bitcast(F32R))
            bg_t = consts.tile([128, 6], F32)
            nc.sync.dma_start(bg_t[:], bg_d[:])
            bo_t = consts.tile([128, 2], F32)
            nc.sync.dma_start(bo_t[:], bo_d[:])
            e_t = consts.tile([8, 256], F32R)
            nc.sync.dma_start(e_t[:], e_d.bitcast(F32R))
            xpad = big.tile([128, XPADF], F32R)
            XV = xpad.rearrange("p (r c) -> p r c", c=HP)  # [128, 39, 38]
            q_t = big.tile([128, 2, Q], F32R)
            k_t = big.tile([128, 2, D], F32R)
            vT_t = big.tile([128, DT, 8, 33], F32R)
            araw = big.tile([128, 2, Q], F32)
            rzr = big.tile([8, Q], F32)

            for _rep in range(rep):
                first = (_rep == 0)
                # ---------------- q / k / vT convolutions ----------------
                if 'conv' in phases or first:
                    with tc.tile_pool(name="cw", bufs=1) as cw:
                        wq_t = cw.tile([128, 9, 256], F32R)
                        nc.sync.dma_start(wq_t[:], wq_d.bitcast(F32R))
                        wk_t = cw.tile([128, 9, 256], F32R)
                        nc.sync.dma_start(wk_t[:], wk_d.bitcast(F32R))
                        wv_t = cw.tile([128, 9, 256], F32R)
                        nc.sync.dma_start(wv_t[:], wv_d.bitcast(F32R))
                        scr = cw.tile([128, XPADF], F32)
                        nc.gpsimd.memset(scr[:], 0.0)
                        nc.vector.tensor_copy(xpad[:], scr[:])
                        nc.sync.dma_start(
                            XV[:, 1:37, 1:37],
                            xc_d.bitcast(F32R).rearrange(
                                "p (r c) -> p r c", c=W))
                        ones_scr = cw.tile([128, DT * 8], F32)
                        nc.gpsimd.memset(ones_scr[:], 1.0)
                        nc.vector.tensor_copy(
                            vT_t[:, :, :, 32],
                            ones_scr.rearrange("p (a b) -> p a b", b=8))
                        # column-shifted compactions of x (stationary matmul
                        # operands need 1D free APs)
                        xdx = []
                        for dx in range(3):
                            t = cw.tile([128, 37 * HK], F32R,
                                        name=f"xdx{dx}")
                            nc.vector.tensor_copy(
                                t.rearrange("p (r c) -> p r c", c=HK),
                                XV[:, 1:38, dx + 1:dx + 1 + HK])
                            xdx.append(t)

                        with tc.tile_pool(name="qkps", bufs=1,
                                          space="PSUM") as qkps:
                            # q conv: both at-tiles in one 6-bank psum tile
                            qp = qkps.tile([128, 2, 3, 512], F32, tag="qk")
                            for at in range(2):
                                for s in range(9):
                                    dy, dx = s // 3, s % 3
                                    lhsT = wq_t[:, s, 128 * at:128 * at + 128]
                                    for ci, (r0, nr) in enumerate(QR_CHUNKS):
                                        rhs = XV[:, r0 + dy:r0 + dy + nr,
                                                 dx:dx + W]
                                        nc.tensor.matmul(
                                            qp[:, at, ci, 0:nr * W], lhsT,
                                            rhs,
                                            start=(s == 0), stop=(s == 8))
                            nc.vector.tensor_copy(
                                q_t.rearrange("p at (ci w) -> p at ci w",
                                              ci=3),
                                qp[:, :, :, 0:432])
                            kp = qkps.tile([128, 2, 3, 512], F32, tag="qk")
                            for at in range(2):
                                for s in range(9):
                                    dy, dx = s // 3, s % 3
                                    lhsT = wk_t[:, s, 128 * at:128 * at + 128]
                                    for ci, (r0, nr) in enumerate(KR_CHUNKS):
                                        rhs = XV[:, r0 + dy + 1:
                                                 r0 + dy + 1 + nr,
                                                 dx + 1:dx + 1 + HK]
                                        nc.tensor.matmul(
                                            kp[:, at, ci, 0:nr * HK], lhsT,
                                            rhs,
                                            start=(s == 0), stop=(s == 8))
                            for at in range(2):
                                nc.vector.tensor_copy(
                                    k_t[:, at, 0:816].rearrange(
                                        "p (ci w) -> p ci w", ci=2),
                                    kp[:, at, 0:2, 0:408])
                                nc.vector.tensor_copy(
                                    k_t[:, at, 816:1156],
                                    kp[:, at, 2, 0:340])

                        # vT conv: pairs of 102-row tiles share one psum tile
                        with (
                            tc.tile_pool(name="vps", bufs=2,
                                         space="PSUM") as vps,
                            tc.tile_pool(name="vtmp", bufs=2) as vtp,
                        ):
                            for tp in range(6):
                                vp = vps.tile([128, 2, 256], F32, tag="v")
                                vt = vtp.tile([128, 2, 256], F32R,
                                              tag="vtmp")
                                for half in range(2):
                                    t2 = 2 * tp + half
                                    d0 = 102 * t2
                                    dn2 = min(102, D - d0)
                                    y0 = d0 // HK
                                    nrows = dn2 // HK
                                    for s in range(9):
                                        dy, dx = s // 3, s % 3
                                        lhsT = xdx[dx][
                                            :, (y0 + dy) * HK:
                                            (y0 + dy + nrows) * HK]
                                        nc.tensor.matmul(
                                            vp[0:dn2, half, :], lhsT,
                                            wv_t[:, s, :],
                                            start=(s == 0), stop=(s == 8))
                                if tp < 5:
                                    nc.vector.tensor_copy(vt[0:102, :, :],
                                                          vp[0:102, :, :])
                                else:
                                    nc.vector.tensor_copy(vt[0:102, 0, :],
                                                          vp[0:102, 0, :])
                                    nc.vector.tensor_copy(vt[0:34, 1, :],
                                                          vp[0:34, 1, :])
                                for half in range(2):
                                    t2 = 2 * tp + half
                                    d0 = 102 * t2
                                    dn2 = min(102, D - d0)
                                    d = d0
                                    src_r = 0
                                    while d < d0 + dn2:
                                        dt, p0 = divmod(d, 128)
                                        ln = min(128 - p0, d0 + dn2 - d)
                                        src = vt[src_r:src_r + ln, half,
                                                 :].rearrange(
                                            "p (h c) -> p h c", c=32)
                                        nc.sync.dma_start(
                                            vT_t[p0:p0 + ln, dt, :, 0:32],
                                            src)
                                        d += ln
                                        src_r += ln

                # ---------------- attention (2 heads row-packed) ----------
                if 'attn' in phases or first:
                    with (
                        tc.tile_pool(name="scps", bufs=1,
                                     space="PSUM") as scps,
                        tc.tile_pool(name="avps", bufs=2,
                                     space="PSUM") as avps,
                        tc.tile_pool(name="exps", bufs=10) as exps,
                        tc.tile_pool(name="avsb", bufs=2) as avsb,
                    ):
                        for pr in range(4):
                            at, jp = pr // 2, pr % 2
                            h0 = 4 * at + 2 * jp
                            expT = []
                            for dt in range(DT):
                                dn = 128 if dt < 9 else D - 9 * 128
                                sp = scps.tile([128, 2, 1536], F32, tag="sc")
                                for hh in range(2):
                                    p0 = 32 * (2 * jp + hh)
                                    for (qo, qn) in Q_CHUNKS:
                                        nc.tensor.matmul(
                                            sp[0:dn, hh, qo:qo + qn],
                                            k_t[p0:p0 + 32, at,
                                                128 * dt:128 * dt + dn],
                                            q_t[p0:p0 + 32, at, qo:qo + qn],
                                            start=True, stop=True,
                                            tile_position=(p0, 0))
                                e = exps.tile([128, 2, 1296], F32R,
                                              tag="expT")
                                nc.scalar.activation(e[0:dn, :, :],
                                                     sp[0:dn, :, 0:1296],
                                                     AF.Exp)
                                expT.append(e)
                            av_sb = avsb.tile([33, 2, Q], F32, tag="avsb")
                            for hh in range(2):
                                h = h0 + hh
                                for (qo, qn) in Q_CHUNKS:
                                    ap_ = avps.tile([33, 512], F32, tag="av")
                                    for dt in range(DT):
                                        dn = 128 if dt < 9 else D - 9 * 128
                                        nc.tensor.matmul(
                                            ap_[0:33, 0:qn],
                                            vT_t[0:dn, dt, h, :],
                                            expT[dt][0:dn, hh, qo:qo + qn],
                                            start=(dt == 0),
                                            stop=(dt == DT - 1))
                                    nc.vector.tensor_copy(
                                        av_sb[0:33, hh, qo:qo + qn],
                                        ap_[0:33, 0:qn])
                            # partition-remap extraction (a strips + Z rows)
                            for hh in range(2):
                                p0 = 32 * (2 * jp + hh)
                                nc.sync.dma_start(
                                    araw[p0:p0 + 32, at, :],
                                    av_sb[0:32, hh, :])
                            nc.sync.dma_start(
                                rzr[h0:h0 + 2, :], av_sb[32:33, :, :])

                # ---------------- normalize + gates + output --------------
                if 'norm' in phases or first:
                    with tc.tile_pool(name="work", bufs=1) as work:
                        rz = work.tile([8, Q], F32R)
                        with nc.allow_low_precision(
                                reason="f32r feed is 19-bit rounded"):
                            nc.vector.reciprocal(rz[:], rzr[:])
                        anorm = work.tile([128, 2, Q], F32R)
                        with tc.tile_pool(name="rps", bufs=1,
                                          space="PSUM") as rps:
                            rp = rps.tile([128, 2, 1536], F32, tag="r")
                            for at in range(2):
                                for (qo, qn) in Q_CHUNKS:
                                    nc.tensor.matmul(
                                        rp[:, at, qo:qo + qn],
                                        e_t[:, 128 * at:128 * at + 128],
                                        rz[:, qo:qo + qn],
                                        start=True, stop=True)
                            nc.vector.tensor_mul(anorm[:], araw[:],
                                                 rp[:, :, 0:1296])

                        sgi = work.tile([128, 2, Q], F32)
                        tgg = work.tile([128, 2, Q], F32)
                        sgo = work.tile([128, 2, Q], F32)
                        gdst = [(sgi, AF.Sigmoid), (tgg, AF.Tanh),
                                (sgo, AF.Sigmoid)]
                        with tc.tile_pool(name="gps", bufs=1,
                                          space="PSUM") as gps:
                            for gi_ in range(3):
                                gp = gps.tile([128, 2, 3, 512], F32, tag="g")
                                for mm in range(2):
                                    m = 2 * gi_ + mm
                                    for ci, (r0, nr) in enumerate(QR_CHUNKS):
                                        qo, qn = r0 * W, nr * W
                                        nc.tensor.matmul(
                                            gp[:, mm, ci, 0:qn],
                                            wga_t[:, 0,
                                                  128 * m:128 * m + 128],
                                            anorm[:, 0, qo:qo + qn],
                                            start=True, stop=False)
                                        nc.tensor.matmul(
                                            gp[:, mm, ci, 0:qn],
                                            wga_t[:, 1,
                                                  128 * m:128 * m + 128],
                                            anorm[:, 1, qo:qo + qn],
                                            start=False, stop=False)
                                        nc.tensor.matmul(
                                            gp[:, mm, ci, 0:qn],
                                            wgx_t[:, 128 * m:128 * m + 128],
                                            XV[:, r0 + 1:r0 + 1 + nr, 1:37],
                                            start=False, stop=True)
                                dst, fn = gdst[gi_]
                                for mm in range(2):
                                    m = 2 * gi_ + mm
                                    nc.scalar.activation(
                                        dst[:, mm, :].rearrange(
                                            "p (a b) -> p a b", a=3),
                                        gp[:, mm, :, 0:432], fn,
                                        bias=bg_t[:, m:m + 1])

                        c_t = work.tile([128, 2, Q], F32)
                        nc.vector.tensor_mul(c_t[:], sgi[:], tgg[:])
                        thc = work.tile([128, 2, Q], F32)
                        nc.scalar.activation(thc[:], c_t[:], AF.Tanh)
                        hn = work.tile([128, 2, Q], F32R)
                        nc.vector.tensor_mul(hn[:], sgo[:], thc[:])

                        out_sb = work.tile([128, 2, Q], F32)
                        with tc.tile_pool(name="ops", bufs=1,
                                          space="PSUM") as ops:
                            op = ops.tile([128, 2, 1536], F32, tag="o")
                            for ot in range(2):
                                for (qo, qn) in Q_CHUNKS:
                                    for rt in range(2):
                                        nc.tensor.matmul(
                                            op[:, ot, qo:qo + qn],
                                            wout_t[:, rt,
                                                   128 * ot:128 * ot + 128],
                                            hn[:, rt, qo:qo + qn],
                                            start=(rt == 0), stop=(rt == 1))
                            for ot in range(2):
                                nc.vector.tensor_scalar_add(
                                    out_sb[:, ot, :], op[:, ot, 0:1296],
                                    bo_t[:, ot:ot + 1])
                            nc.sync.dma_start(o_d[0:128, :], out_sb[:, 0, :])
                            nc.sync.dma_start(o_d[128:256, :],
                                              out_sb[:, 1, :])

    nc.compile()
    return nc


def _prep(inputs):
    f8 = np.float64
    x = np.asarray(inputs['x'], np.float32)
    Wi = np.asarray(inputs['w_in'], f8)[:, :, 0, 0]           # [R, I]
    b_in = np.asarray(inputs['b_in'], f8)
    assert np.allclose(b_in, 0.0), "nonzero b_in unsupported by this build"

    def fold3(w):  # [A,R,3,3] x [R,I] -> [128 i, 9 s, 256 a]
        we = np.einsum('arst,ri->aist', np.asarray(w, f8), Wi)
        return np.ascontiguousarray(
            we.transpose(1, 2, 3, 0).reshape(I, 9, A).astype(np.float32))

    wq = fold3(inputs['w_qx'])
    wk = fold3(inputs['w_kx'])
    wv = fold3(inputs['w_vx'])

    keep = np.r_[0:R, 2 * R:4 * R]                            # gi, gg, go
    Wga = np.asarray(inputs['w_ga'], f8)[:, :, 0, 0][keep]    # [G, A]
    wga = np.ascontiguousarray(
        Wga.T.reshape(2, 128, G).astype(np.float32))
    Wgx = np.asarray(inputs['w_gx'], f8)[:, :, 0, 0][keep]    # [G, R]
    Wgx_eff = Wgx @ Wi                                        # [G, I]
    wgx = np.ascontiguousarray(Wgx_eff.T.astype(np.float32))  # [128, G]
    b_eff = np.asarray(inputs['b_g'], f8)[keep] + Wgx @ b_in
    bg = np.ascontiguousarray(
        b_eff.reshape(6, 128).T.astype(np.float32))           # [128, 6]
    Wo = np.asarray(inputs['w_out'], f8)[:, :, 0, 0]          # [O, R]
    wout = np.ascontiguousarray(
        Wo.T.reshape(2, 128, 256).astype(np.float32))
    bo = np.ascontiguousarray(
        np.asarray(inputs['b_out'], f8).reshape(2, 128).T.astype(np.float32))
    e = np.zeros((8, 256), np.float32)
    for h in range(8):
        at, j = h // 4, h % 4
        e[h, 128 * at + 32 * j:128 * at + 32 * j + 32] = 1.0

    shared = dict(wq=wq, wk=wk, wv=wv, wga=wga, wgx=wgx, wout=wout,
                  bg=bg, bo=bo, e=e)
    return [dict(shared, xc=np.ascontiguousarray(x[n].reshape(128, Q)))
            for n in range(N)]


def get_nc(rep=1, phases=('conv', 'attn', 'norm')):
    key = ('nc', rep, tuple(phases))
    if key not in _CACHE:
        _CACHE[key] = _build(rep, phases)
    return _CACHE[key]


def kernel(**inputs):
    nc = get_nc()
    in_maps = _prep(inputs)
    res = run_bass_kernel_spmd(nc, in_maps, core_ids=list(range(N)))
    out = np.stack([res.results[n]['o'].reshape(O, H, W) for n in range(N)])
    return out.astype(np.float32)

